# revision 20
# baseline (speedup 1.0000x reference)
"""Trainium2 Bass kernel for nn_Encoder_50852412785097 (sparse_attention).

Math (validated against the jax reference to ~1e-6):
  Per (b, h):
    Q = X wQ_h, K = X wK_h, V = X wV_h              (X = inputs[b], [S, D])
    e = (Q K^T) / sqrt(D)
    rr_t = den - cumsum_t(exp(e))  # den = masked row sum of exp(e)
    decay = exp((theta^2/den) * (t_j - t_i) * rr)
    u = exp(e * decay)             # unnormalized second softmax
    out_h = ((u @ V) / sum_j u) @ wO_h
  out[b] = sum_h out_h

Sharding: 16 (b, h) pairs over 8 cores -> core c handles b = c//4,
heads {2*(c%4), 2*(c%4)+1}. Weights replicated; host sums the 4 partial
outputs per batch.

v4 structure:
 - custom DVE op SCAN_WW_ANT (registered into concourse.dve_ops at
   import): out = (in1-s0)*(1 - cumsum(in0)*s1), i.e. the rr suffix-sum,
   the (t_j-t_i) product and the 1/den normalization fused in one DVE
   pass (decay exp then just scales by theta^2).
 - causal mask via PE matmul accumulate (identb.T @ maskb).
 - sarr (= es*decay) is produced in bf16 and PE-TRANSPOSED; the u-exp
   reads the transposed PSUM and writes uT directly (no DVE PSUM->SBUF
   copy for the AV operand). den2 comes for free: the AV stationary vx
   carries a ones row (65th column) and wox carries a unit column, so
   po2x[:, h, 64] = den2 lands via the output projection matmul.
 - software pipelining: per-unit stages A (QK+escpy+exp+den), B
   (scan_ww+decay+mul), C (transpose+u-exp+AV+wo) are emitted
   interleaved (A(k) | B(k-1) | C(k-2)) so each engine's in-order
   stream always has independent work to fill dependency stalls.
"""

import os
import sys

import numpy as np

B, S, H, D = 2, 2048, 8, 64
P = 128
NT = S // P  # 16 row tiles
NH = 2  # heads per core
NCORES = 8
MASK_VAL = -1e30
DX = D + 1  # AV stationary width incl. ones row


def _import_concourse():
    try:
        import concourse.bass  # noqa: F401
    except ImportError:
        for p in ("/opt/trn_rl_repo", "/root/.axon_site/_ro/trn_rl_repo"):
            if os.path.isdir(p) and p not in sys.path:
                sys.path.insert(0, p)
        import concourse.bass  # noqa: F401


_SCAN_WW = None


def _register_scan_ww():
    """Fused DVE op: out = (in1 - s0) * (1 - cumsum(in0) * s1).

    With in0=exp(es), in1=t_j, s0=t_i, s1=1/den this is
    (t_j - t_i) * (den - cumsum) / den = ww/den in ONE DVE pass
    (replaces tensor_tensor_scan + scalar_tensor_tensor + the spp
    scalar combine)."""
    global _SCAN_WW
    if _SCAN_WW is not None:
        return _SCAN_WW
    from concourse import dve_ops
    from concourse.dve_spec import (
        Spec, Src0, Src1, C0, C1, One, AluOp, scan, lower,
    )
    from concourse.dve_uop import DveOpSpec

    name = "SCAN_WW_ANT"
    for op in dve_ops.OPS:
        if op.name == name:
            _SCAN_WW = op
            return op

    def ref(in0, in1, s0, s1, imm2):
        return (in1 - s0) * (
            1.0 - np.cumsum(in0.astype(np.float32), axis=1) * s1
        )

    spec = Spec(
        body=(Src1 - C0) * (One - scan(AluOp.ADD, Src0) * C1),
        reference=ref,
    )
    shas = {}
    for ver in ("v3", "v4"):
        try:
            shas[ver] = DveOpSpec(
                name=name, opcode=1, uops=lower(spec, ver=ver), rd1_en=True
            ).sha(ver)
        except Exception:
            pass
    op = dve_ops.DveOp(name, spec, subdim=False, uops_sha=shas)
    row = max(dve_ops._SUB_OPCODE_FOR_NAME.values()) + 1
    assert row < 0x20
    dve_ops.OPS.append(op)
    dve_ops._SUB_OPCODE_FOR_NAME[name] = row
    dve_ops.CUSTOM_DVE_SPECS[name] = spec
    _SCAN_WW = op
    return op


def build_nc(ch=1024, es_split=False, db=2, dc=4, esbf=True):
    """Build the SPMD single-core program (same on all 8 cores)."""
    _import_concourse()
    import concourse.bass as bass
    import concourse.bacc as bacc
    from concourse import mybir
    from concourse.tile import TileContext

    scan_ww = _register_scan_ww()

    f32 = mybir.dt.float32
    bf16 = mybir.dt.bfloat16
    Alu = mybir.AluOpType
    Act = mybir.ActivationFunctionType

    CH = ch

    nc = bacc.Bacc("TRN2", target_bir_lowering=False, debug=False)

    # --- external I/O (per core) ---
    xT_h = nc.dram_tensor("xT", [D, S], f32, kind="ExternalInput")     # X^T
    tsj_h = nc.dram_tensor("tsj", [1, S], f32, kind="ExternalInput")   # t_j row
    tsi_h = nc.dram_tensor("tsi", [P, NT], f32, kind="ExternalInput")  # t_i cols
    wq_h = nc.dram_tensor("wq", [D, NH * D], f32, kind="ExternalInput")
    wk_h = nc.dram_tensor("wk", [D, NH * D], f32, kind="ExternalInput")
    wv_h = nc.dram_tensor("wv", [D, NH * D], f32, kind="ExternalInput")
    wo_h = nc.dram_tensor("wo", [D, NH * D], f32, kind="ExternalInput")
    th_h = nc.dram_tensor("th", [1, 1], f32, kind="ExternalInput")
    y_h = nc.dram_tensor("y", [S, D], f32, kind="ExternalOutput")

    # --- NEFF-embedded constants ---
    mask_np = np.triu(np.ones((P, P), np.float32), k=1) * np.float32(MASK_VAL)
    mask_dram = nc.inline_tensor(mask_np, name="maskc")
    ident_dram = nc.inline_tensor(np.eye(P, dtype=np.float32), name="identc")

    with TileContext(nc) as tc:
        from contextlib import ExitStack

        with ExitStack() as ctx:
            consts = ctx.enter_context(tc.tile_pool(name="consts", bufs=1))

            def load(shape, handle_ap, via, name, dt=f32):
                stage = consts.tile(shape, f32, tag=f"stg_{name}")
                nc.gpsimd.dma_start(out=stage, in_=handle_ap)
                dst = consts.tile(shape, dt, tag=name)
                via(dst, stage)
                return dst, stage

            # PE-consumed: staged via DVE
            maskb, _ = load([P, P], mask_dram[:, :], nc.vector.tensor_copy,
                            "maskb", dt=bf16)
            identb, _ = load([P, P], ident_dram[:, :], nc.vector.tensor_copy,
                             "identb", dt=bf16)
            xT, _ = load([D, S], xT_h[:, :], nc.vector.tensor_copy, "xT")
            xTb = consts.tile([D, S], bf16, tag="xTb")
            nc.vector.tensor_copy(xTb, xT)
            wq, _ = load([D, NH * D], wq_h[:, :], nc.vector.tensor_copy,
                         "wq", dt=bf16)
            wk, _ = load([D, NH * D], wk_h[:, :], nc.vector.tensor_copy,
                         "wk", dt=bf16)
            wv, _ = load([D, NH * D], wv_h[:, :], nc.vector.tensor_copy,
                         "wv", dt=bf16)

            # wo stays f32 stage; wox (bf16) is assembled below
            stg_wo = consts.tile([D, NH * D], f32, tag="stg_wo")
            nc.gpsimd.dma_start(out=stg_wo, in_=wo_h[:, :])

            # DVE-consumed (fused scan op): staged via GPSIMD
            tsj_ap = tsj_h[:, :]
            tsj_b = bass.AP(
                tensor=tsj_ap.tensor, offset=tsj_ap.offset,
                ap=[[0, P], list(tsj_ap.ap[-1])],
            )
            tsj, _ = load([P, S], tsj_b, nc.gpsimd.tensor_copy, "tsj")
            tsi, _ = load([P, NT], tsi_h[:, :], nc.gpsimd.tensor_copy, "tsi")

            # theta broadcast -> th2 = theta^2
            thb = consts.tile([P, 1], f32)
            th_ap = th_h[:, :]
            th_b = bass.AP(
                tensor=th_ap.tensor, offset=th_ap.offset,
                ap=[[0, P], list(th_ap.ap[-1])],
            )
            nc.gpsimd.dma_start(out=thb, in_=th_b)
            th2 = consts.tile([P, 1], f32)
            nc.vector.tensor_mul(th2, thb, thb)

            # wox: [DX, NH*DX] bf16 = per head [[wo_h, 0], [0, 1]] so that
            # po2x[:, h, D] = den2 falls out of the projection matmul.
            wox = consts.tile([DX, NH * DX], bf16, tag="wox")
            nc.vector.memset(wox, 0.0)
            for h in range(NH):
                nc.vector.tensor_copy(
                    wox[0:D, h * DX: h * DX + D],
                    stg_wo[:, h * D:(h + 1) * D])
                nc.vector.memset(wox[D:DX, h * DX + D: h * DX + DX], 1.0)

            # --- projections: qt (scaled by 1/8), kt: [64, NH*S];
            #     vx: [128, NT*NH*DX] (st-major, ones row per block) ---
            qt = consts.tile([D, NH * S], bf16)
            kt = consts.tile([D, NH * S], bf16)
            vx = consts.tile([P, NT * NH * DX], bf16, tag="vx")
            vx_ap = vx[:, :]
            vones_ap = bass.AP(
                tensor=vx_ap.tensor, offset=vx_ap.offset + D,
                ap=[list(vx_ap.ap[0]), [DX, NT * NH]],
            )
            nc.gpsimd.memset(vones_ap, 1.0)
            with tc.tile_pool(name="psetup", bufs=2, space="PSUM") as psetup:
                # wq is prescaled by 1/8 host-side, so qt is a plain copy
                for h in range(NH):
                    for sc in range(S // 512):
                        pq = psetup.tile([D, 512], f32, tag="pq")
                        nc.tensor.matmul(
                            pq, lhsT=wq[:, h * D:(h + 1) * D],
                            rhs=xTb[:, 512 * sc:512 * (sc + 1)],
                            start=True, stop=True,
                        )
                        nc.scalar.copy(
                            qt[:, h * S + 512 * sc: h * S + 512 * (sc + 1)],
                            pq)
                        pk = psetup.tile([D, 512], f32, tag="pk")
                        nc.tensor.matmul(
                            pk, lhsT=wk[:, h * D:(h + 1) * D],
                            rhs=xTb[:, 512 * sc:512 * (sc + 1)],
                            start=True, stop=True,
                        )
                        nc.vector.tensor_copy(
                            kt[:, h * S + 512 * sc: h * S + 512 * (sc + 1)],
                            pk)
                for st in range(NT):
                    pv = psetup.tile([P, NH * D], f32, tag="pv")
                    nc.tensor.matmul(
                        pv, lhsT=xTb[:, P * st:P * (st + 1)], rhs=wv[:, :],
                        start=True, stop=True,
                    )
                    for h in range(NH):
                        cpv = nc.scalar.copy if h == 0 else nc.vector.tensor_copy
                        cpv(vx[:, (st * NH + h) * DX:(st * NH + h) * DX + D],
                            pv[:, h * D:(h + 1) * D])

            # --- main pipeline (software-pipelined A/B/C stages) ---
            work = ctx.enter_context(tc.tile_pool(name="work",
                                                  bufs=max(db + 2, dc - db + 2)))
            work2 = ctx.enter_context(tc.tile_pool(name="work2", bufs=2))
            small = ctx.enter_context(tc.tile_pool(name="small", bufs=8))
            ppe = ctx.enter_context(tc.tile_pool(name="ppe", bufs=2,
                                                 space="PSUM"))
            ppt = ctx.enter_context(
                tc.tile_pool(name="ppt", bufs=2, space="PSUM"))
            pprT = ctx.enter_context(tc.tile_pool(name="pprT", bufs=1,
                                                  space="PSUM"))
            ppo = ctx.enter_context(tc.tile_pool(name="ppo", bufs=1,
                                                 space="PSUM"))

            units = [(ti, h) for ti in range(NT) for h in range(NH)]
            UN = len(units)
            po2x_by_ti = {}

            def emit_A(k):
                """QK -> PSUM, drain es to SBUF, exp full-W (accum den)."""
                ti, h = units[k]
                W = P * (ti + 1)
                nch = (W + CH - 1) // CH
                def escpy(dst, src, _c=[0]):
                    eng = (nc.scalar.copy if (es_split and _c[0] % 2 == 1)
                           else nc.vector.tensor_copy)
                    _c[0] += 1
                    eng(dst, src)
                qrow = qt[:, h * S + P * ti: h * S + P * (ti + 1)]
                es = work.tile([P, S], bf16 if esbf else f32, tag="es")
                scr = work.tile([P, S], f32, tag="scr")
                for c in range(nch):
                    lo, hi = CH * c, min(W, CH * (c + 1))
                    cols = hi - lo
                    pe = ppe.tile([P, CH], f32, tag="pe")
                    j0 = lo
                    while j0 < hi:
                        j1 = min(hi, j0 + 512)
                        nc.tensor.matmul(
                            pe[:, j0 - lo:j1 - lo], lhsT=qrow,
                            rhs=kt[:, h * S + j0: h * S + j1],
                            start=True, stop=(j1 != W),
                        )
                        j0 = j1
                    if hi == W:
                        nc.tensor.matmul(
                            pe[:, cols - P:cols], lhsT=identb, rhs=maskb,
                            start=False, stop=True,
                        )
                    escpy(es[:, lo:hi], pe[:, :cols])
                den = small.tile([P, 1], f32, tag="den")
                nc.scalar.activation(scr[:, :W], es[:, :W], Act.Exp,
                                     accum_out=den)
                rden = small.tile([P, 1], f32, tag="rden")
                nc.vector.reciprocal(rden, den)
                return {"ti": ti, "h": h, "W": W, "es": es, "scr": scr,
                        "rden": rden}

            def emit_B(st):
                """scan_ww (in place), decay exp (in place), sarr mul."""
                ti, h, W = st["ti"], st["h"], st["W"]
                es, scr = st["es"], st["scr"]
                t_i = tsi[:, ti:ti + 1]
                # ping-pong scr -> rr -> scr (no in-place ops)
                rr = work2.tile([P, S], f32, tag="rr")
                nc.vector._custom_dve(
                    scan_ww, out=rr[:, :W], in0=scr[:, :W],
                    in1=tsj[:, :W], s0=t_i, s1=st["rden"],
                )
                nc.scalar.activation(scr[:, :W], rr[:, :W], Act.Exp,
                                     scale=th2)
                sarrb = work.tile([P, S], bf16, tag="sarrb")
                nc.gpsimd.tensor_mul(sarrb[:, :W], scr[:, :W], es[:, :W])
                st["sarrb"] = sarrb

            def emit_C(st):
                """Transpose sarr, u-exp from PSUM -> uT, AV+wo matmuls."""
                ti, h, W = st["ti"], st["h"], st["W"]
                sarrb = st["sarrb"]
                njb = ti + 1
                uT = work2.tile([P, S], bf16, tag="uT")
                for g0 in range(0, njb, 8):
                    gn = min(8, njb - g0)
                    pt = ppt.tile([P, 8 * P], bf16, tag="pt")
                    for q in range(gn):
                        nc.tensor.transpose(
                            pt[:, q * P:(q + 1) * P],
                            sarrb[:, (g0 + q) * P:(g0 + q + 1) * P], identb)
                    nc.scalar.activation(uT[:, g0 * P:(g0 + gn) * P],
                                         pt[:, :gn * P], Act.Exp)
                prT = pprT.tile([DX, P], f32, tag="prT")
                for jb in range(njb):
                    nc.tensor.matmul(
                        prT,
                        lhsT=vx[:, (jb * NH + h) * DX:(jb * NH + h + 1) * DX],
                        rhs=uT[:, jb * P:(jb + 1) * P],
                        start=(jb == 0), stop=(jb == ti),
                    )
                rT = small.tile([DX, P], bf16, tag="rT")
                nc.vector.tensor_copy(rT, prT)
                if h == 0:
                    po2x_by_ti[ti] = ppo.tile([P, NH, DX], f32, tag="po2x",
                                              name="po2x")
                po2x = po2x_by_ti[ti]
                nc.tensor.matmul(po2x[:, h, :], lhsT=rT,
                                 rhs=wox[:, h * DX:(h + 1) * DX],
                                 start=True, stop=True)
                if h == NH - 1:
                    # y = po2x[0]/den2_0 + po2x[1]/den2_1 ; DMA out
                    r0 = small.tile([P, 1], f32, tag="r0")
                    nc.vector.reciprocal(r0, po2x[:, 0, D:DX])
                    r1 = small.tile([P, 1], f32, tag="r1")
                    nc.vector.reciprocal(r1, po2x[:, 1, D:DX])
                    t0 = small.tile([P, D], f32, tag="t0")
                    nc.vector.tensor_scalar(t0, po2x[:, 0, 0:D], scalar1=r0,
                                            scalar2=None, op0=Alu.mult)
                    ys = small.tile([P, D], f32, tag="ys")
                    nc.vector.scalar_tensor_tensor(
                        ys, in0=po2x[:, 1, 0:D], scalar=r1, in1=t0,
                        op0=Alu.mult, op1=Alu.add,
                    )
                    nc.sync.dma_start(out=y_h[P * ti:P * (ti + 1), :],
                                      in_=ys)
                    del po2x_by_ti[ti]

            states = {}
            for k in range(UN + dc):
                if k < UN:
                    states[k] = emit_A(k)
                if db <= k < UN + db:
                    emit_B(states[k - db])
                if dc <= k:
                    emit_C(states[k - dc])
                    del states[k - dc]

    if not nc.is_finalized():
        nc.finalize()
    return nc


_NC_CACHE = {}

KERNEL_FLAGS = {}


def _get_nc():
    key = tuple(sorted(KERNEL_FLAGS.items()))
    if key not in _NC_CACHE:
        _NC_CACHE[key] = build_nc(**KERNEL_FLAGS)
    return _NC_CACHE[key]


def make_in_maps(inputs, timestamp, wQ, wK, wV, wO, theta):
    x = np.asarray(inputs, np.float32)
    t = np.asarray(timestamp).astype(np.float32)
    wQ = np.asarray(wQ, np.float32) * np.float32(0.125)  # fold 1/sqrt(D)
    wK = np.asarray(wK, np.float32)
    wV = np.asarray(wV, np.float32)
    wO = np.asarray(wO, np.float32)
    theta = np.asarray(theta, np.float32)

    in_maps = []
    for c in range(NCORES):
        b = c // 4
        h0 = NH * (c % 4)
        in_maps.append({
            "xT": np.ascontiguousarray(x[b].T),
            "tsj": np.ascontiguousarray(t[b][None, :]),
            "tsi": np.ascontiguousarray(t[b].reshape(NT, P).T),
            "wq": np.ascontiguousarray(np.concatenate([wQ[h0], wQ[h0 + 1]], axis=1)),
            "wk": np.ascontiguousarray(np.concatenate([wK[h0], wK[h0 + 1]], axis=1)),
            "wv": np.ascontiguousarray(np.concatenate([wV[h0], wV[h0 + 1]], axis=1)),
            "wo": np.ascontiguousarray(np.concatenate(
                [wO[h0 * D:(h0 + 1) * D], wO[(h0 + 1) * D:(h0 + 2) * D]], axis=1)),
            "th": np.ascontiguousarray(theta.reshape(1, 1)),
        })
    return in_maps


def kernel(inputs, timestamp, wQ, wK, wV, wO, theta, _trace=False, _trace_kwargs=None):
    _import_concourse()
    from concourse.bass_utils import run_bass_kernel_spmd

    nc = _get_nc()
    in_maps = make_in_maps(inputs, timestamp, wQ, wK, wV, wO, theta)
    res = run_bass_kernel_spmd(
        nc, in_maps, list(range(NCORES)),
        trace=_trace, **(_trace_kwargs or {}),
    )
    out = np.zeros((B, S, D), np.float32)
    for c in range(NCORES):
        out[c // 4] += res.results[c]["y"]
    if _trace:
        return out, res
    return out


if __name__ == "__main__":
    nc = build_nc()
    print("built ok")


# revision 22
# speedup vs baseline: 1.1697x; 1.1697x over previous
"""Trainium2 Bass kernel for nn_Encoder_50852412785097 (sparse_attention).

Math (validated against the jax reference to ~1e-6):
  Per (b, h):
    Q = X wQ_h, K = X wK_h, V = X wV_h              (X = inputs[b], [S, D])
    e = (Q K^T) / sqrt(D)
    rr_t = den - cumsum_t(exp(e))  # den = masked row sum of exp(e)
    decay = exp((theta^2/den) * (t_j - t_i) * rr)
    u = exp(e * decay)             # unnormalized second softmax
    out_h = ((u @ V) / sum_j u) @ wO_h
  out[b] = sum_h out_h

Sharding: 16 (b, h) pairs over 8 cores -> core c handles b = c//4,
heads {2*(c%4), 2*(c%4)+1}. Weights replicated; host sums the 4 partial
outputs per batch.

v4 structure:
 - custom DVE op SCAN_WW_ANT (registered into concourse.dve_ops at
   import): out = (in1-s0)*(1 - cumsum(in0)*s1), i.e. the rr suffix-sum,
   the (t_j-t_i) product and the 1/den normalization fused in one DVE
   pass (decay exp then just scales by theta^2).
 - causal mask via PE matmul accumulate (identb.T @ maskb).
 - sarr (= es*decay) is produced in bf16 and PE-TRANSPOSED; the u-exp
   reads the transposed PSUM and writes uT directly (no DVE PSUM->SBUF
   copy for the AV operand). den2 comes for free: the AV stationary vx
   carries a ones row (65th column) and wox carries a unit column, so
   po2x[:, h, 64] = den2 lands via the output projection matmul.
 - software pipelining: per-unit stages A (QK+escpy+exp+den), B
   (scan_ww+decay+mul), C (transpose+u-exp+AV+wo) are emitted
   interleaved (A(k) | B(k-db) | C(k-dc), default db=2/dc=4) so each
   engine's in-order stream always has independent work to fill
   dependency stalls. es is drained to SBUF in bf16.

Measured on HW: 178132 ns (baseline 299940 ns), absmax-rel err 3.3e-3.
"""

import os
import sys

import numpy as np

B, S, H, D = 2, 2048, 8, 64
P = 128
NT = S // P  # 16 row tiles
NH = 2  # heads per core
NCORES = 8
MASK_VAL = -1e30
DX = D + 1  # AV stationary width incl. ones row


def _import_concourse():
    try:
        import concourse.bass  # noqa: F401
    except ImportError:
        for p in ("/opt/trn_rl_repo", "/root/.axon_site/_ro/trn_rl_repo"):
            if os.path.isdir(p) and p not in sys.path:
                sys.path.insert(0, p)
        import concourse.bass  # noqa: F401


_SCAN_WW = None


def _register_scan_ww():
    """Fused DVE op: out = (in1 - s0) * (1 - cumsum(in0) * s1).

    With in0=exp(es), in1=t_j, s0=t_i, s1=1/den this is
    (t_j - t_i) * (den - cumsum) / den = ww/den in ONE DVE pass
    (replaces tensor_tensor_scan + scalar_tensor_tensor + the spp
    scalar combine)."""
    global _SCAN_WW
    if _SCAN_WW is not None:
        return _SCAN_WW
    from concourse import dve_ops
    from concourse.dve_spec import (
        Spec, Src0, Src1, C0, C1, One, AluOp, scan, lower,
    )
    from concourse.dve_uop import DveOpSpec

    name = "SCAN_WW_ANT"
    for op in dve_ops.OPS:
        if op.name == name:
            _SCAN_WW = op
            return op

    def ref(in0, in1, s0, s1, imm2):
        return (in1 - s0) * (
            1.0 - np.cumsum(in0.astype(np.float32), axis=1) * s1
        )

    spec = Spec(
        body=(Src1 - C0) * (One - scan(AluOp.ADD, Src0) * C1),
        reference=ref,
    )
    shas = {}
    for ver in ("v3", "v4"):
        try:
            shas[ver] = DveOpSpec(
                name=name, opcode=1, uops=lower(spec, ver=ver), rd1_en=True
            ).sha(ver)
        except Exception:
            pass
    op = dve_ops.DveOp(name, spec, subdim=False, uops_sha=shas)
    row = max(dve_ops._SUB_OPCODE_FOR_NAME.values()) + 1
    assert row < 0x20
    dve_ops.OPS.append(op)
    dve_ops._SUB_OPCODE_FOR_NAME[name] = row
    dve_ops.CUSTOM_DVE_SPECS[name] = spec
    _SCAN_WW = op
    return op


def build_nc(ch=2048, es_split=False, db=2, dc=4, esbf=True, ilv=True):
    """Build the SPMD single-core program (same on all 8 cores)."""
    _import_concourse()
    import concourse.bass as bass
    import concourse.bacc as bacc
    from concourse import mybir
    from concourse.tile import TileContext

    scan_ww = _register_scan_ww()

    f32 = mybir.dt.float32
    bf16 = mybir.dt.bfloat16
    Alu = mybir.AluOpType
    Act = mybir.ActivationFunctionType

    CH = ch

    nc = bacc.Bacc("TRN2", target_bir_lowering=False, debug=False)

    # --- external I/O (per core) ---
    xT_h = nc.dram_tensor("xT", [D, S], f32, kind="ExternalInput")     # X^T
    tsj_h = nc.dram_tensor("tsj", [1, S], f32, kind="ExternalInput")   # t_j row
    tsi_h = nc.dram_tensor("tsi", [P, NT], f32, kind="ExternalInput")  # t_i cols
    wq_h = nc.dram_tensor("wq", [D, NH * D], f32, kind="ExternalInput")
    wk_h = nc.dram_tensor("wk", [D, NH * D], f32, kind="ExternalInput")
    wv_h = nc.dram_tensor("wv", [D, NH * D], f32, kind="ExternalInput")
    wo_h = nc.dram_tensor("wo", [D, NH * D], f32, kind="ExternalInput")
    th_h = nc.dram_tensor("th", [1, 1], f32, kind="ExternalInput")
    y_h = nc.dram_tensor("y", [S, D], f32, kind="ExternalOutput")

    # --- NEFF-embedded constants ---
    mask_np = np.triu(np.ones((P, P), np.float32), k=1) * np.float32(MASK_VAL)
    mask_dram = nc.inline_tensor(mask_np, name="maskc")
    ident_dram = nc.inline_tensor(np.eye(P, dtype=np.float32), name="identc")

    with TileContext(nc) as tc:
        from contextlib import ExitStack

        with ExitStack() as ctx:
            consts = ctx.enter_context(tc.tile_pool(name="consts", bufs=1))

            def load(shape, handle_ap, via, name, dt=f32):
                stage = consts.tile(shape, f32, tag=f"stg_{name}")
                nc.gpsimd.dma_start(out=stage, in_=handle_ap)
                dst = consts.tile(shape, dt, tag=name)
                via(dst, stage)
                return dst, stage

            # PE-consumed: staged via DVE
            maskb, _ = load([P, P], mask_dram[:, :], nc.vector.tensor_copy,
                            "maskb", dt=bf16)
            identb, _ = load([P, P], ident_dram[:, :], nc.vector.tensor_copy,
                             "identb", dt=bf16)
            xT, _ = load([D, S], xT_h[:, :], nc.vector.tensor_copy, "xT")
            xTb = consts.tile([D, S], bf16, tag="xTb")
            nc.vector.tensor_copy(xTb, xT)
            wq, _ = load([D, NH * D], wq_h[:, :], nc.vector.tensor_copy,
                         "wq", dt=bf16)
            wk, _ = load([D, NH * D], wk_h[:, :], nc.vector.tensor_copy,
                         "wk", dt=bf16)
            wv, _ = load([D, NH * D], wv_h[:, :], nc.vector.tensor_copy,
                         "wv", dt=bf16)

            # wo stays f32 stage; wox (bf16) is assembled below
            stg_wo = consts.tile([D, NH * D], f32, tag="stg_wo")
            nc.gpsimd.dma_start(out=stg_wo, in_=wo_h[:, :])

            # DVE-consumed (fused scan op): staged via GPSIMD
            tsj_ap = tsj_h[:, :]
            tsj_b = bass.AP(
                tensor=tsj_ap.tensor, offset=tsj_ap.offset,
                ap=[[0, P], list(tsj_ap.ap[-1])],
            )
            tsj, _ = load([P, S], tsj_b, nc.gpsimd.tensor_copy, "tsj")
            tsi, _ = load([P, NT], tsi_h[:, :], nc.gpsimd.tensor_copy, "tsi")

            # theta broadcast -> th2 = theta^2
            thb = consts.tile([P, 1], f32)
            th_ap = th_h[:, :]
            th_b = bass.AP(
                tensor=th_ap.tensor, offset=th_ap.offset,
                ap=[[0, P], list(th_ap.ap[-1])],
            )
            nc.gpsimd.dma_start(out=thb, in_=th_b)
            th2 = consts.tile([P, 1], f32)
            nc.vector.tensor_mul(th2, thb, thb)

            # wox: [DX, NH*DX] bf16 = per head [[wo_h, 0], [0, 1]] so that
            # po2x[:, h, D] = den2 falls out of the projection matmul.
            wox = consts.tile([DX, NH * DX], bf16, tag="wox")
            nc.vector.memset(wox, 0.0)
            for h in range(NH):
                nc.vector.tensor_copy(
                    wox[0:D, h * DX: h * DX + D],
                    stg_wo[:, h * D:(h + 1) * D])
                nc.vector.memset(wox[D:DX, h * DX + D: h * DX + DX], 1.0)

            # --- projections: qt (scaled by 1/8), kt: [64, NH*S];
            #     vx: [128, NT*NH*DX] (st-major, ones row per block) ---
            qt = consts.tile([D, NH * S], bf16)
            kt = consts.tile([D, NH * S], bf16)
            vx = consts.tile([P, NT * NH * DX], bf16, tag="vx")
            vx_ap = vx[:, :]
            vones_ap = bass.AP(
                tensor=vx_ap.tensor, offset=vx_ap.offset + D,
                ap=[list(vx_ap.ap[0]), [DX, NT * NH]],
            )
            nc.gpsimd.memset(vones_ap, 1.0)
            with tc.tile_pool(name="psetup", bufs=2, space="PSUM") as psetup:
                # wq is prescaled by 1/8 host-side, so qt is a plain copy
                for h in range(NH):
                    for sc in range(S // 512):
                        pq = psetup.tile([D, 512], f32, tag="pq")
                        nc.tensor.matmul(
                            pq, lhsT=wq[:, h * D:(h + 1) * D],
                            rhs=xTb[:, 512 * sc:512 * (sc + 1)],
                            start=True, stop=True,
                        )
                        nc.scalar.copy(
                            qt[:, h * S + 512 * sc: h * S + 512 * (sc + 1)],
                            pq)
                        pk = psetup.tile([D, 512], f32, tag="pk")
                        nc.tensor.matmul(
                            pk, lhsT=wk[:, h * D:(h + 1) * D],
                            rhs=xTb[:, 512 * sc:512 * (sc + 1)],
                            start=True, stop=True,
                        )
                        nc.vector.tensor_copy(
                            kt[:, h * S + 512 * sc: h * S + 512 * (sc + 1)],
                            pk)
                for st in range(NT):
                    pv = psetup.tile([P, NH * D], f32, tag="pv")
                    nc.tensor.matmul(
                        pv, lhsT=xTb[:, P * st:P * (st + 1)], rhs=wv[:, :],
                        start=True, stop=True,
                    )
                    for h in range(NH):
                        cpv = nc.scalar.copy if h == 0 else nc.vector.tensor_copy
                        cpv(vx[:, (st * NH + h) * DX:(st * NH + h) * DX + D],
                            pv[:, h * D:(h + 1) * D])

            # --- main pipeline (software-pipelined A/B/C stages) ---
            work = ctx.enter_context(tc.tile_pool(name="work",
                                                  bufs=max(db + 2, dc - db + 2)))
            work2 = ctx.enter_context(tc.tile_pool(name="work2", bufs=2))
            small = ctx.enter_context(tc.tile_pool(name="small", bufs=8))
            ppe = ctx.enter_context(tc.tile_pool(
                name="ppe", bufs=(1 if CH == 2048 else 2), space="PSUM"))
            ppt = ctx.enter_context(
                tc.tile_pool(name="ppt", bufs=2, space="PSUM"))
            pprT = ctx.enter_context(tc.tile_pool(name="pprT", bufs=1,
                                                  space="PSUM"))
            ppo = ctx.enter_context(tc.tile_pool(name="ppo", bufs=1,
                                                 space="PSUM"))

            # big/small interleave keeps per-round engine loads uniform
            # and makes the drain tail cheap; (ti,h0),(ti,h1) stay adjacent
            if ilv:
                ti_order = []
                lo_t, hi_t = 0, NT - 1
                while lo_t <= hi_t:
                    ti_order.append(hi_t)
                    if lo_t < hi_t:
                        ti_order.append(lo_t)
                    hi_t -= 1
                    lo_t += 1
            else:
                ti_order = list(range(NT))
            units = [(ti, h) for ti in ti_order for h in range(NH)]
            UN = len(units)
            po2x_by_ti = {}

            def emit_A(k):
                """QK -> PSUM, drain es to SBUF, exp full-W (accum den)."""
                ti, h = units[k]
                W = P * (ti + 1)
                nch = (W + CH - 1) // CH
                def escpy(dst, src, _c=[0]):
                    eng = (nc.scalar.copy if (es_split and _c[0] % 2 == 1)
                           else nc.vector.tensor_copy)
                    _c[0] += 1
                    eng(dst, src)
                qrow = qt[:, h * S + P * ti: h * S + P * (ti + 1)]
                es = work.tile([P, S], bf16 if esbf else f32, tag="es")
                scr = work.tile([P, S], f32, tag="scr")
                for c in range(nch):
                    lo, hi = CH * c, min(W, CH * (c + 1))
                    cols = hi - lo
                    pe = ppe.tile([P, CH], f32, tag="pe")
                    j0 = lo
                    while j0 < hi:
                        j1 = min(hi, j0 + 512)
                        nc.tensor.matmul(
                            pe[:, j0 - lo:j1 - lo], lhsT=qrow,
                            rhs=kt[:, h * S + j0: h * S + j1],
                            start=True, stop=(j1 != W),
                        )
                        j0 = j1
                    if hi == W:
                        nc.tensor.matmul(
                            pe[:, cols - P:cols], lhsT=identb, rhs=maskb,
                            start=False, stop=True,
                        )
                    escpy(es[:, lo:hi], pe[:, :cols])
                den = small.tile([P, 1], f32, tag="den")
                nc.scalar.activation(scr[:, :W], es[:, :W], Act.Exp,
                                     accum_out=den)
                rden = small.tile([P, 1], f32, tag="rden")
                nc.vector.reciprocal(rden, den)
                return {"ti": ti, "h": h, "W": W, "es": es, "scr": scr,
                        "rden": rden}

            def emit_B(st):
                """scan_ww (in place), decay exp (in place), sarr mul."""
                ti, h, W = st["ti"], st["h"], st["W"]
                es, scr = st["es"], st["scr"]
                t_i = tsi[:, ti:ti + 1]
                # ping-pong scr -> rr -> scr (no in-place ops)
                rr = work2.tile([P, S], f32, tag="rr")
                nc.vector._custom_dve(
                    scan_ww, out=rr[:, :W], in0=scr[:, :W],
                    in1=tsj[:, :W], s0=t_i, s1=st["rden"],
                )
                nc.scalar.activation(scr[:, :W], rr[:, :W], Act.Exp,
                                     scale=th2)
                sarrb = work.tile([P, S], bf16, tag="sarrb")
                nc.gpsimd.tensor_mul(sarrb[:, :W], scr[:, :W], es[:, :W])
                st["sarrb"] = sarrb

            def emit_C(st):
                """Transpose sarr, u-exp from PSUM -> uT, AV+wo matmuls."""
                ti, h, W = st["ti"], st["h"], st["W"]
                sarrb = st["sarrb"]
                njb = ti + 1
                uT = work2.tile([P, S], bf16, tag="uT")
                for g0 in range(0, njb, 8):
                    gn = min(8, njb - g0)
                    pt = ppt.tile([P, 8 * P], bf16, tag="pt")
                    for q in range(gn):
                        nc.tensor.transpose(
                            pt[:, q * P:(q + 1) * P],
                            sarrb[:, (g0 + q) * P:(g0 + q + 1) * P], identb)
                    nc.scalar.activation(uT[:, g0 * P:(g0 + gn) * P],
                                         pt[:, :gn * P], Act.Exp)
                prT = pprT.tile([DX, P], f32, tag="prT")
                for jb in range(njb):
                    nc.tensor.matmul(
                        prT,
                        lhsT=vx[:, (jb * NH + h) * DX:(jb * NH + h + 1) * DX],
                        rhs=uT[:, jb * P:(jb + 1) * P],
                        start=(jb == 0), stop=(jb == ti),
                    )
                rT = small.tile([DX, P], bf16, tag="rT")
                nc.vector.tensor_copy(rT, prT)
                if h == 0:
                    po2x_by_ti[ti] = ppo.tile([P, NH, DX], f32, tag="po2x",
                                              name="po2x")
                po2x = po2x_by_ti[ti]
                nc.tensor.matmul(po2x[:, h, :], lhsT=rT,
                                 rhs=wox[:, h * DX:(h + 1) * DX],
                                 start=True, stop=True)
                if h == NH - 1:
                    # y = po2x[0]/den2_0 + po2x[1]/den2_1 ; DMA out
                    r01 = small.tile([P, NH], f32, tag="r01")
                    nc.vector.reciprocal(r01, po2x[:, :, D])
                    r0, r1 = r01[:, 0:1], r01[:, 1:2]
                    t0 = small.tile([P, D], f32, tag="t0")
                    nc.vector.tensor_scalar(t0, po2x[:, 0, 0:D], scalar1=r0,
                                            scalar2=None, op0=Alu.mult)
                    ys = small.tile([P, D], f32, tag="ys")
                    nc.vector.scalar_tensor_tensor(
                        ys, in0=po2x[:, 1, 0:D], scalar=r1, in1=t0,
                        op0=Alu.mult, op1=Alu.add,
                    )
                    nc.sync.dma_start(out=y_h[P * ti:P * (ti + 1), :],
                                      in_=ys)
                    del po2x_by_ti[ti]

            states = {}
            for k in range(UN + dc):
                if k < UN:
                    states[k] = emit_A(k)
                if db <= k < UN + db:
                    emit_B(states[k - db])
                if dc <= k:
                    emit_C(states[k - dc])
                    del states[k - dc]

    if not nc.is_finalized():
        nc.finalize()
    return nc


_NC_CACHE = {}

KERNEL_FLAGS = {}


def _get_nc():
    key = tuple(sorted(KERNEL_FLAGS.items()))
    if key not in _NC_CACHE:
        _NC_CACHE[key] = build_nc(**KERNEL_FLAGS)
    return _NC_CACHE[key]


def make_in_maps(inputs, timestamp, wQ, wK, wV, wO, theta):
    x = np.asarray(inputs, np.float32)
    t = np.asarray(timestamp).astype(np.float32)
    wQ = np.asarray(wQ, np.float32) * np.float32(0.125)  # fold 1/sqrt(D)
    wK = np.asarray(wK, np.float32)
    wV = np.asarray(wV, np.float32)
    wO = np.asarray(wO, np.float32)
    theta = np.asarray(theta, np.float32)

    in_maps = []
    for c in range(NCORES):
        b = c // 4
        h0 = NH * (c % 4)
        in_maps.append({
            "xT": np.ascontiguousarray(x[b].T),
            "tsj": np.ascontiguousarray(t[b][None, :]),
            "tsi": np.ascontiguousarray(t[b].reshape(NT, P).T),
            "wq": np.ascontiguousarray(np.concatenate([wQ[h0], wQ[h0 + 1]], axis=1)),
            "wk": np.ascontiguousarray(np.concatenate([wK[h0], wK[h0 + 1]], axis=1)),
            "wv": np.ascontiguousarray(np.concatenate([wV[h0], wV[h0 + 1]], axis=1)),
            "wo": np.ascontiguousarray(np.concatenate(
                [wO[h0 * D:(h0 + 1) * D], wO[(h0 + 1) * D:(h0 + 2) * D]], axis=1)),
            "th": np.ascontiguousarray(theta.reshape(1, 1)),
        })
    return in_maps


def kernel(inputs, timestamp, wQ, wK, wV, wO, theta, _trace=False, _trace_kwargs=None):
    _import_concourse()
    from concourse.bass_utils import run_bass_kernel_spmd

    nc = _get_nc()
    in_maps = make_in_maps(inputs, timestamp, wQ, wK, wV, wO, theta)
    res = run_bass_kernel_spmd(
        nc, in_maps, list(range(NCORES)),
        trace=_trace, **(_trace_kwargs or {}),
    )
    out = np.zeros((B, S, D), np.float32)
    for c in range(NCORES):
        out[c // 4] += res.results[c]["y"]
    if _trace:
        return out, res
    return out


if __name__ == "__main__":
    nc = build_nc()
    print("built ok")


# revision 23
# speedup vs baseline: 1.2450x; 1.0644x over previous
"""Trainium2 Bass kernel for nn_Encoder_50852412785097 (sparse_attention).

Math (validated against the jax reference to ~1e-6):
  Per (b, h):
    Q = X wQ_h, K = X wK_h, V = X wV_h              (X = inputs[b], [S, D])
    e = (Q K^T) / sqrt(D)
    rr_t = den - cumsum_t(exp(e))  # den = masked row sum of exp(e)
    decay = exp((theta^2/den) * (t_j - t_i) * rr)
    u = exp(e * decay)             # unnormalized second softmax
    out_h = ((u @ V) / sum_j u) @ wO_h
  out[b] = sum_h out_h

Sharding: 16 (b, h) pairs over 8 cores -> core c handles b = c//4,
heads {2*(c%4), 2*(c%4)+1}. Weights replicated; host sums the 4 partial
outputs per batch.

v4 structure:
 - custom DVE op SCAN_WW_ANT (registered into concourse.dve_ops at
   import): out = (in1-s0)*(1 - cumsum(in0)*s1), i.e. the rr suffix-sum,
   the (t_j-t_i) product and the 1/den normalization fused in one DVE
   pass (decay exp then just scales by theta^2).
 - causal mask via PE matmul accumulate (identb.T @ maskb).
 - sarr (= es*decay) is produced in bf16 and PE-TRANSPOSED; the u-exp
   reads the transposed PSUM and writes uT directly (no DVE PSUM->SBUF
   copy for the AV operand). den2 comes for free: the AV stationary vx
   carries a ones row (65th column) and wox carries a unit column, so
   po2x[:, h, 64] = den2 lands via the output projection matmul.
 - software pipelining: per-unit stages A (QK+escpy+exp+den), B
   (scan_ww+decay+mul), C (transpose+u-exp+AV+wo) are emitted
   interleaved (A(k) | B(k-db) | C(k-dc), default db=2/dc=4) so each
   engine's in-order stream always has independent work to fill
   dependency stalls. es is drained to SBUF in bf16.

Measured on HW: ~178 us (baseline 299.9 us), absmax-rel err 3.3e-3.
"""

import os
import sys

import numpy as np

B, S, H, D = 2, 2048, 8, 64
P = 128
NT = S // P  # 16 row tiles
NH = 2  # heads per core
NCORES = 8
MASK_VAL = -1e30
DX = D + 1  # AV stationary width incl. ones row


def _import_concourse():
    try:
        import concourse.bass  # noqa: F401
    except ImportError:
        for p in ("/opt/trn_rl_repo", "/root/.axon_site/_ro/trn_rl_repo"):
            if os.path.isdir(p) and p not in sys.path:
                sys.path.insert(0, p)
        import concourse.bass  # noqa: F401


_SCAN_WW = None


def _register_scan_ww():
    """Fused DVE op: out = (in1 - s0) * (1 - cumsum(in0) * s1).

    With in0=exp(es), in1=t_j, s0=t_i, s1=1/den this is
    (t_j - t_i) * (den - cumsum) / den = ww/den in ONE DVE pass
    (replaces tensor_tensor_scan + scalar_tensor_tensor + the spp
    scalar combine)."""
    global _SCAN_WW
    if _SCAN_WW is not None:
        return _SCAN_WW
    from concourse import dve_ops
    from concourse.dve_spec import (
        Spec, Src0, Src1, C0, C1, One, AluOp, scan, lower,
    )
    from concourse.dve_uop import DveOpSpec

    name = "SCAN_WW_ANT"
    for op in dve_ops.OPS:
        if op.name == name:
            _SCAN_WW = op
            return op

    def ref(in0, in1, s0, s1, imm2):
        return (in1 - s0) * (
            1.0 - np.cumsum(in0.astype(np.float32), axis=1) * s1
        )

    spec = Spec(
        body=(Src1 - C0) * (One - scan(AluOp.ADD, Src0) * C1),
        reference=ref,
    )
    shas = {}
    for ver in ("v3", "v4"):
        try:
            shas[ver] = DveOpSpec(
                name=name, opcode=1, uops=lower(spec, ver=ver), rd1_en=True
            ).sha(ver)
        except Exception:
            pass
    op = dve_ops.DveOp(name, spec, subdim=False, uops_sha=shas)
    row = max(dve_ops._SUB_OPCODE_FOR_NAME.values()) + 1
    assert row < 0x20
    dve_ops.OPS.append(op)
    dve_ops._SUB_OPCODE_FOR_NAME[name] = row
    dve_ops.CUSTOM_DVE_SPECS[name] = spec
    _SCAN_WW = op
    return op


def build_nc(ch=1024, es_split=False, db=2, dc=4, esbf=True, ilv=False):
    """Build the SPMD single-core program (same on all 8 cores)."""
    _import_concourse()
    import concourse.bass as bass
    import concourse.bacc as bacc
    from concourse import mybir
    from concourse.tile import TileContext

    scan_ww = _register_scan_ww()

    f32 = mybir.dt.float32
    bf16 = mybir.dt.bfloat16
    Alu = mybir.AluOpType
    Act = mybir.ActivationFunctionType

    CH = ch

    nc = bacc.Bacc("TRN2", target_bir_lowering=False, debug=False)

    # --- external I/O (per core) ---
    xT_h = nc.dram_tensor("xT", [D, S], f32, kind="ExternalInput")     # X^T
    tsj_h = nc.dram_tensor("tsj", [1, S], f32, kind="ExternalInput")   # t_j row
    tsi_h = nc.dram_tensor("tsi", [P, NT], f32, kind="ExternalInput")  # t_i cols
    wq_h = nc.dram_tensor("wq", [D, NH * D], f32, kind="ExternalInput")
    wk_h = nc.dram_tensor("wk", [D, NH * D], f32, kind="ExternalInput")
    wv_h = nc.dram_tensor("wv", [D, NH * D], f32, kind="ExternalInput")
    wo_h = nc.dram_tensor("wo", [D, NH * D], f32, kind="ExternalInput")
    th_h = nc.dram_tensor("th", [1, 1], f32, kind="ExternalInput")
    y_h = nc.dram_tensor("y", [S, D], f32, kind="ExternalOutput")

    # --- NEFF-embedded constants ---
    mask_np = np.triu(np.ones((P, P), np.float32), k=1) * np.float32(MASK_VAL)
    mask_dram = nc.inline_tensor(mask_np, name="maskc")
    ident_dram = nc.inline_tensor(np.eye(P, dtype=np.float32), name="identc")

    with TileContext(nc) as tc:
        from contextlib import ExitStack

        with ExitStack() as ctx:
            consts = ctx.enter_context(tc.tile_pool(name="consts", bufs=1))

            def load(shape, handle_ap, via, name, dt=f32):
                stage = consts.tile(shape, f32, tag=f"stg_{name}")
                nc.gpsimd.dma_start(out=stage, in_=handle_ap)
                dst = consts.tile(shape, dt, tag=name)
                via(dst, stage)
                return dst, stage

            # PE-consumed: staged via DVE
            maskb, _ = load([P, P], mask_dram[:, :], nc.vector.tensor_copy,
                            "maskb", dt=bf16)
            identb, _ = load([P, P], ident_dram[:, :], nc.vector.tensor_copy,
                             "identb", dt=bf16)
            xT, _ = load([D, S], xT_h[:, :], nc.vector.tensor_copy, "xT")
            xTb = consts.tile([D, S], bf16, tag="xTb")
            nc.vector.tensor_copy(xTb, xT)
            wq, _ = load([D, NH * D], wq_h[:, :], nc.vector.tensor_copy,
                         "wq", dt=bf16)
            wk, _ = load([D, NH * D], wk_h[:, :], nc.vector.tensor_copy,
                         "wk", dt=bf16)
            wv, _ = load([D, NH * D], wv_h[:, :], nc.vector.tensor_copy,
                         "wv", dt=bf16)

            # wo stays f32 stage; wox (bf16) is assembled below
            stg_wo = consts.tile([D, NH * D], f32, tag="stg_wo")
            nc.gpsimd.dma_start(out=stg_wo, in_=wo_h[:, :])

            # DVE-consumed (fused scan op): staged via GPSIMD
            tsj_ap = tsj_h[:, :]
            tsj_b = bass.AP(
                tensor=tsj_ap.tensor, offset=tsj_ap.offset,
                ap=[[0, P], list(tsj_ap.ap[-1])],
            )
            tsj, _ = load([P, S], tsj_b, nc.gpsimd.tensor_copy, "tsj")
            tsi, _ = load([P, NT], tsi_h[:, :], nc.gpsimd.tensor_copy, "tsi")

            # theta broadcast -> th2 = theta^2
            thb = consts.tile([P, 1], f32)
            th_ap = th_h[:, :]
            th_b = bass.AP(
                tensor=th_ap.tensor, offset=th_ap.offset,
                ap=[[0, P], list(th_ap.ap[-1])],
            )
            nc.gpsimd.dma_start(out=thb, in_=th_b)
            th2 = consts.tile([P, 1], f32)
            nc.vector.tensor_mul(th2, thb, thb)

            # wox: [DX, NH*DX] bf16 = per head [[wo_h, 0], [0, 1]] so that
            # po2x[:, h, D] = den2 falls out of the projection matmul.
            wox = consts.tile([DX, NH * DX], bf16, tag="wox")
            nc.vector.memset(wox, 0.0)
            for h in range(NH):
                nc.vector.tensor_copy(
                    wox[0:D, h * DX: h * DX + D],
                    stg_wo[:, h * D:(h + 1) * D])
                nc.vector.memset(wox[D:DX, h * DX + D: h * DX + DX], 1.0)

            # --- projections: qt (scaled by 1/8), kt: [64, NH*S];
            #     vx: [128, NT*NH*DX] (st-major, ones row per block) ---
            qt = consts.tile([D, NH * S], bf16)
            kt = consts.tile([D, NH * S], bf16)
            vx = consts.tile([P, NT * NH * DX], bf16, tag="vx")
            vx_ap = vx[:, :]
            vones_ap = bass.AP(
                tensor=vx_ap.tensor, offset=vx_ap.offset + D,
                ap=[list(vx_ap.ap[0]), [DX, NT * NH]],
            )
            nc.gpsimd.memset(vones_ap, 1.0)
            with tc.tile_pool(name="psetup", bufs=2, space="PSUM") as psetup:
                # wq is prescaled by 1/8 host-side, so qt is a plain copy
                for h in range(NH):
                    for sc in range(S // 512):
                        pq = psetup.tile([D, 512], f32, tag="pq")
                        nc.tensor.matmul(
                            pq, lhsT=wq[:, h * D:(h + 1) * D],
                            rhs=xTb[:, 512 * sc:512 * (sc + 1)],
                            start=True, stop=True,
                        )
                        nc.scalar.copy(
                            qt[:, h * S + 512 * sc: h * S + 512 * (sc + 1)],
                            pq)
                        pk = psetup.tile([D, 512], f32, tag="pk")
                        nc.tensor.matmul(
                            pk, lhsT=wk[:, h * D:(h + 1) * D],
                            rhs=xTb[:, 512 * sc:512 * (sc + 1)],
                            start=True, stop=True,
                        )
                        nc.vector.tensor_copy(
                            kt[:, h * S + 512 * sc: h * S + 512 * (sc + 1)],
                            pk)
                for st in range(NT):
                    pv = psetup.tile([P, NH * D], f32, tag="pv")
                    nc.tensor.matmul(
                        pv, lhsT=xTb[:, P * st:P * (st + 1)], rhs=wv[:, :],
                        start=True, stop=True,
                    )
                    for h in range(NH):
                        cpv = nc.scalar.copy if h == 0 else nc.vector.tensor_copy
                        cpv(vx[:, (st * NH + h) * DX:(st * NH + h) * DX + D],
                            pv[:, h * D:(h + 1) * D])

            # --- main pipeline (software-pipelined A/B/C stages) ---
            work = ctx.enter_context(tc.tile_pool(name="work",
                                                  bufs=max(db + 2, dc - db + 2)))
            work2 = ctx.enter_context(tc.tile_pool(name="work2", bufs=2))
            small = ctx.enter_context(tc.tile_pool(name="small", bufs=8))
            ppe = ctx.enter_context(tc.tile_pool(
                name="ppe", bufs=(1 if CH == 2048 else 2), space="PSUM"))
            ppt = ctx.enter_context(
                tc.tile_pool(name="ppt", bufs=2, space="PSUM"))
            pprT = ctx.enter_context(tc.tile_pool(name="pprT", bufs=1,
                                                  space="PSUM"))
            ppo = ctx.enter_context(tc.tile_pool(name="ppo", bufs=1,
                                                 space="PSUM"))

            # big/small interleave keeps per-round engine loads uniform
            # and makes the drain tail cheap; (ti,h0),(ti,h1) stay adjacent
            if ilv:
                ti_order = []
                lo_t, hi_t = 0, NT - 1
                while lo_t <= hi_t:
                    ti_order.append(hi_t)
                    if lo_t < hi_t:
                        ti_order.append(lo_t)
                    hi_t -= 1
                    lo_t += 1
            else:
                ti_order = list(range(NT))
            units = [(ti, h) for ti in ti_order for h in range(NH)]
            UN = len(units)
            po2x_by_ti = {}

            def emit_A(k):
                """QK -> PSUM, drain es to SBUF, exp full-W (accum den)."""
                ti, h = units[k]
                W = P * (ti + 1)
                nch = (W + CH - 1) // CH
                def escpy(dst, src, _c=[0]):
                    eng = (nc.scalar.copy if (es_split and _c[0] % 2 == 1)
                           else nc.vector.tensor_copy)
                    _c[0] += 1
                    eng(dst, src)
                qrow = qt[:, h * S + P * ti: h * S + P * (ti + 1)]
                es = work.tile([P, S], bf16 if esbf else f32, tag="es")
                scr = work.tile([P, S], f32, tag="scr")
                for c in range(nch):
                    lo, hi = CH * c, min(W, CH * (c + 1))
                    cols = hi - lo
                    pe = ppe.tile([P, CH], f32, tag="pe")
                    j0 = lo
                    while j0 < hi:
                        j1 = min(hi, j0 + 512)
                        nc.tensor.matmul(
                            pe[:, j0 - lo:j1 - lo], lhsT=qrow,
                            rhs=kt[:, h * S + j0: h * S + j1],
                            start=True, stop=(j1 != W),
                        )
                        j0 = j1
                    if hi == W:
                        nc.tensor.matmul(
                            pe[:, cols - P:cols], lhsT=identb, rhs=maskb,
                            start=False, stop=True,
                        )
                    escpy(es[:, lo:hi], pe[:, :cols])
                den = small.tile([P, 1], f32, tag="den")
                nc.scalar.activation(scr[:, :W], es[:, :W], Act.Exp,
                                     accum_out=den)
                rden = small.tile([P, 1], f32, tag="rden")
                nc.vector.reciprocal(rden, den)
                return {"ti": ti, "h": h, "W": W, "es": es, "scr": scr,
                        "rden": rden}

            def emit_B(st):
                """scan_ww (in place), decay exp (in place), sarr mul."""
                ti, h, W = st["ti"], st["h"], st["W"]
                es, scr = st["es"], st["scr"]
                t_i = tsi[:, ti:ti + 1]
                # ping-pong scr -> rr -> scr (no in-place ops)
                rr = work2.tile([P, S], f32, tag="rr")
                nc.vector._custom_dve(
                    scan_ww, out=rr[:, :W], in0=scr[:, :W],
                    in1=tsj[:, :W], s0=t_i, s1=st["rden"],
                )
                nc.scalar.activation(scr[:, :W], rr[:, :W], Act.Exp,
                                     scale=th2)
                sarrb = work.tile([P, S], bf16, tag="sarrb")
                nc.gpsimd.tensor_mul(sarrb[:, :W], scr[:, :W], es[:, :W])
                st["sarrb"] = sarrb

            def emit_C(st):
                """Transpose sarr, u-exp from PSUM -> uT, AV+wo matmuls."""
                ti, h, W = st["ti"], st["h"], st["W"]
                sarrb = st["sarrb"]
                njb = ti + 1
                uT = work2.tile([P, S], bf16, tag="uT")
                for g0 in range(0, njb, 8):
                    gn = min(8, njb - g0)
                    pt = ppt.tile([P, 8 * P], bf16, tag="pt")
                    for q in range(gn):
                        nc.tensor.transpose(
                            pt[:, q * P:(q + 1) * P],
                            sarrb[:, (g0 + q) * P:(g0 + q + 1) * P], identb)
                    nc.scalar.activation(uT[:, g0 * P:(g0 + gn) * P],
                                         pt[:, :gn * P], Act.Exp)
                prT = pprT.tile([DX, P], f32, tag="prT")
                for jb in range(njb):
                    nc.tensor.matmul(
                        prT,
                        lhsT=vx[:, (jb * NH + h) * DX:(jb * NH + h + 1) * DX],
                        rhs=uT[:, jb * P:(jb + 1) * P],
                        start=(jb == 0), stop=(jb == ti),
                    )
                rT = small.tile([DX, P], bf16, tag="rT")
                nc.vector.tensor_copy(rT, prT)
                if h == 0:
                    po2x_by_ti[ti] = ppo.tile([P, NH, DX], f32, tag="po2x",
                                              name="po2x")
                po2x = po2x_by_ti[ti]
                nc.tensor.matmul(po2x[:, h, :], lhsT=rT,
                                 rhs=wox[:, h * DX:(h + 1) * DX],
                                 start=True, stop=True)
                if h == NH - 1:
                    # y = po2x[0]/den2_0 + po2x[1]/den2_1 ; DMA out
                    r01 = small.tile([P, NH], f32, tag="r01")
                    nc.vector.reciprocal(r01, po2x[:, :, D])
                    r0, r1 = r01[:, 0:1], r01[:, 1:2]
                    t0 = small.tile([P, D], f32, tag="t0")
                    nc.vector.tensor_scalar(t0, po2x[:, 0, 0:D], scalar1=r0,
                                            scalar2=None, op0=Alu.mult)
                    ys = small.tile([P, D], f32, tag="ys")
                    nc.vector.scalar_tensor_tensor(
                        ys, in0=po2x[:, 1, 0:D], scalar=r1, in1=t0,
                        op0=Alu.mult, op1=Alu.add,
                    )
                    nc.sync.dma_start(out=y_h[P * ti:P * (ti + 1), :],
                                      in_=ys)
                    del po2x_by_ti[ti]

            states = {}
            for k in range(UN + dc):
                if k < UN:
                    states[k] = emit_A(k)
                if db <= k < UN + db:
                    emit_B(states[k - db])
                if dc <= k:
                    emit_C(states[k - dc])
                    del states[k - dc]

    if not nc.is_finalized():
        nc.finalize()
    return nc


_NC_CACHE = {}

KERNEL_FLAGS = {}


def _get_nc():
    key = tuple(sorted(KERNEL_FLAGS.items()))
    if key not in _NC_CACHE:
        _NC_CACHE[key] = build_nc(**KERNEL_FLAGS)
    return _NC_CACHE[key]


def make_in_maps(inputs, timestamp, wQ, wK, wV, wO, theta):
    x = np.asarray(inputs, np.float32)
    t = np.asarray(timestamp).astype(np.float32)
    wQ = np.asarray(wQ, np.float32) * np.float32(0.125)  # fold 1/sqrt(D)
    wK = np.asarray(wK, np.float32)
    wV = np.asarray(wV, np.float32)
    wO = np.asarray(wO, np.float32)
    theta = np.asarray(theta, np.float32)

    in_maps = []
    for c in range(NCORES):
        b = c // 4
        h0 = NH * (c % 4)
        in_maps.append({
            "xT": np.ascontiguousarray(x[b].T),
            "tsj": np.ascontiguousarray(t[b][None, :]),
            "tsi": np.ascontiguousarray(t[b].reshape(NT, P).T),
            "wq": np.ascontiguousarray(np.concatenate([wQ[h0], wQ[h0 + 1]], axis=1)),
            "wk": np.ascontiguousarray(np.concatenate([wK[h0], wK[h0 + 1]], axis=1)),
            "wv": np.ascontiguousarray(np.concatenate([wV[h0], wV[h0 + 1]], axis=1)),
            "wo": np.ascontiguousarray(np.concatenate(
                [wO[h0 * D:(h0 + 1) * D], wO[(h0 + 1) * D:(h0 + 2) * D]], axis=1)),
            "th": np.ascontiguousarray(theta.reshape(1, 1)),
        })
    return in_maps


def kernel(inputs, timestamp, wQ, wK, wV, wO, theta, _trace=False, _trace_kwargs=None):
    _import_concourse()
    from concourse.bass_utils import run_bass_kernel_spmd

    nc = _get_nc()
    in_maps = make_in_maps(inputs, timestamp, wQ, wK, wV, wO, theta)
    res = run_bass_kernel_spmd(
        nc, in_maps, list(range(NCORES)),
        trace=_trace, **(_trace_kwargs or {}),
    )
    out = np.zeros((B, S, D), np.float32)
    for c in range(NCORES):
        out[c // 4] += res.results[c]["y"]
    if _trace:
        return out, res
    return out


if __name__ == "__main__":
    nc = build_nc()
    print("built ok")
